# revision 65
# baseline (speedup 1.0000x reference)
"""Trainium2 Bass kernel for nn_AttentionConv (sparse_attention).

Sharding: pure data-parallel over batch B=8 across the 8 NeuronCores
(core i processes batch element i; parameters replicated; no collectives).

Per-core pipeline (channels on partitions, N=2048 points, K=16 neighbors):
  local half:   k_hat/v_hat = Wk/Wv @ x per-kk into PSUM (PE, bf16);
                s = (k_hat + rel_k)*q fused on DVE (scalar_tensor_tensor,
                PSUM source); e = exp(s) on ACT (bf16); ev = e*v_hat (DVE,
                PSUM source); num/den = bf16 tree adds (DVE + GPSIMD);
                out_loc = num * recip_approx(den).
  pooling:      host-side channel permutation [0,2,..,126,1,3,..,127] turns
                the adjacent-channel-pair max into max(rows 0:64, 64:128);
                the upper half is realigned with a small SBUF->SBUF DMA.
  memory half:  scoresT[j,n'] = k_mem_tile.T @ q_p (PE); E = exp (ACT,
                PSUM->SBUF bf16); [num;den] += [v_memT | 1].T @ E (PE) --
                the fused ones column makes the softmax denominator a free
                by-product, so no partition reductions are needed.
  epilogue:     ck = Wck @ num; out = out_loc + gamma'*ck*bcast(1/den) + beta
                (dividing by den commutes with the channel matmul); the
                output DMA un-permutes channels via a strided DRAM view.
"""

import sys

sys.path.insert(0, "/opt/trn_rl_repo")
import os  # noqa: E402

import antenv  # noqa: E402

_SHIM = "/root/problem/_shim/antenv"
if os.path.isdir(_SHIM) and _SHIM not in antenv.__path__:
    antenv.__path__.append(_SHIM)

import numpy as np  # noqa: E402
import ml_dtypes  # noqa: E402
import concourse.tile as tile  # noqa: E402
from concourse import bacc, mybir  # noqa: E402
from concourse.bass_utils import run_bass_kernel_spmd  # noqa: E402
from concourse.masks import make_identity  # noqa: E402

F32 = mybir.dt.float32
BF16 = mybir.dt.bfloat16
AF = mybir.ActivationFunctionType
ALU = mybir.AluOpType

B, CIN, COUT, N, K, MEM = 8, 128, 128, 2048, 16, 64
EPS = 1e-5
NB = 512          # n-chunk for the local-attention stage
NNB = N // NB     # 4 chunks
KG = 4            # kk-granularity of x streaming
JT = 128          # j-tile width for memory attention
NJT = 2 * N // JT  # 32 j-tiles
FILL_DISABLE = False

_CACHED = {}


def build_nc():
    nc = bacc.Bacc()

    x_d = nc.dram_tensor("x", [CIN, K * N], F32, kind="ExternalInput")  # (c,kk,n)
    absx_d = nc.dram_tensor("abs_x", [CIN // 2, N], F32, kind="ExternalInput")
    kprev_d = nc.dram_tensor("kprev", [MEM, N], F32, kind="ExternalInput")
    vprev_d = nc.dram_tensor("vprev", [MEM, N], F32, kind="ExternalInput")
    wqT_d = nc.dram_tensor("wqT", [CIN // 2, COUT], BF16, kind="ExternalInput")
    wkT_d = nc.dram_tensor("wkT", [CIN, COUT], BF16, kind="ExternalInput")
    wvT_d = nc.dram_tensor("wvT", [CIN, COUT], BF16, kind="ExternalInput")
    rel_d = nc.dram_tensor("rel", [COUT, K], F32, kind="ExternalInput")
    relp_d = nc.dram_tensor("relp", [1, K * COUT], BF16, kind="ExternalInput")
    ohp_d = nc.dram_tensor("ohp", [1, 512], BF16, kind="ExternalInput")
    wckT_d = nc.dram_tensor("wckT", [MEM, COUT], BF16, kind="ExternalInput")
    gamma_d = nc.dram_tensor("gamma", [COUT, 1], F32, kind="ExternalInput")
    beta_d = nc.dram_tensor("beta", [COUT, 1], F32, kind="ExternalInput")

    out_d = nc.dram_tensor("out", [COUT, N], F32, kind="ExternalOutput")
    kp_d = nc.dram_tensor("kp", [MEM, N], F32, kind="ExternalOutput")
    vp_d = nc.dram_tensor("vp", [MEM, N], F32, kind="ExternalOutput")

    x_v = x_d.rearrange("c (k n) -> c k n", k=K)

    with tile.TileContext(nc) as tc:
        import contextlib

        ctx = contextlib.ExitStack()
        with ctx:
            const = ctx.enter_context(tc.tile_pool(name="const", bufs=1))
            pers = ctx.enter_context(tc.tile_pool(name="pers", bufs=1))
            mm = ctx.enter_context(tc.tile_pool(name="mm", bufs=3, space="PSUM"))
            nps = ctx.enter_context(tc.tile_pool(name="nps", bufs=1, space="PSUM"))
            pctx = contextlib.ExitStack()
            ptmp = pctx.enter_context(tc.tile_pool(name="ptmp", bufs=1))

            wq_sb = const.tile([CIN // 2, COUT], BF16, tag="wq")
            nc.sync.dma_start(wq_sb[:], wqT_d[:])
            wk_sb = const.tile([CIN, COUT], BF16, tag="wk")
            nc.sync.dma_start(wk_sb[:], wkT_d[:])
            wv_sb = const.tile([CIN, COUT], BF16, tag="wv")
            nc.sync.dma_start(wv_sb[:], wvT_d[:])
            relp_sb = const.tile([1, K * COUT], BF16, tag="relp")
            nc.sync.dma_start(relp_sb[:], relp_d[:])
            ohpair = const.tile([1, NB], BF16, tag="ohpair")
            nc.sync.dma_start(ohpair[:], ohp_d[:])
            wck_sb = const.tile([MEM, COUT], BF16, tag="wck")
            nc.sync.dma_start(wck_sb[:], wckT_d[:])
            gamma_sb = const.tile([COUT, 1], F32, tag="gamma")
            nc.sync.dma_start(gamma_sb[:], gamma_d[:])
            beta_sb = const.tile([COUT, 1], F32, tag="beta")
            nc.sync.dma_start(beta_sb[:], beta_d[:])
            ident = const.tile([MEM, MEM], BF16, tag="ident")
            make_identity(nc, ident[:])

            q_sb = pers.tile([COUT, N], BF16, tag="q")
            qp_bf = pers.tile([MEM, N], BF16, tag="qp")
            kmem_bf = pers.tile([MEM, 2 * N], BF16, tag="kmem")
            vmem_bf = pers.tile([MEM, 2 * N], BF16, tag="vmem")
            out_loc = pers.tile([COUT, N], F32, tag="outloc")
            num_bf = pers.tile([MEM, N], BF16, tag="numbf")
            den_m = pers.tile([1, N], F32, tag="denm")
            rden_m = den_m  # reciprocal computed in-place
            num_psA = nps.tile([MEM + 1, N // 2], F32, tag="numA")
            num_ps_h = [num_psA, num_psA]  # h=1 reuses after h=0 evacuation


            # ---- q path + kk=0 prologue (pools, v_memT) ----
            absx_bf = ptmp.tile([CIN // 2, N], BF16, tag="absx")
            nc.gpsimd.dma_start(absx_bf[:], absx_d[:])  # cast
            x0 = ptmp.tile([CIN, N], BF16, tag="x0")
            nc.gpsimd.dma_start(x0[:], x_v[:, 0, :])  # kk=0 slice, cast
            kprevf = ptmp.tile([MEM, N], F32, tag="kprevf")
            nc.sync.dma_start(kprevf[:], kprev_d[:])
            vprevf = ptmp.tile([MEM, N], F32, tag="vprevf")
            nc.sync.dma_start(vprevf[:], vprev_d[:])
            nc.scalar.copy(kmem_bf[:, 0:N], kprevf[:])
            nc.scalar.copy(vmem_bf[:, 0:N], vprevf[:])

            for c in range(4):
                csl = slice(c * NB, (c + 1) * NB)
                ps = mm.tile([COUT, 2, NB], F32, tag="mm", name=f"pq{c}")
                nc.tensor.matmul(ps[:, 0, :], wq_sb[:], absx_bf[:, csl],
                                 start=True, stop=True)
                nc.vector.tensor_copy(q_sb[:, csl], ps[:, 0, :])
            q_hi = ptmp.tile([MEM, N], BF16, tag="qhi")
            nc.sync.dma_start(q_hi[:], q_sb[MEM:COUT, :])
            nc.vector.tensor_tensor(qp_bf[:], q_sb[0:MEM, :], q_hi[:], op=ALU.max)

            kall = pers.tile([COUT, N], BF16, tag="kall")
            vall = pers.tile([COUT, N], BF16, tag="vall")

            def prologue_kv():
                for c in range(4):
                    csl = slice(c * NB, (c + 1) * NB)
                    ps = mm.tile([COUT, 2, NB], F32, tag="mm", name=f"pkv{c}")
                    nc.tensor.matmul(ps[:, 0, :], wk_sb[:], x0[:, csl],
                                     start=True, stop=True)
                    nc.tensor.matmul(ps[:, 1, :], wv_sb[:], x0[:, csl],
                                     start=True, stop=True)
                    nc.scalar.copy(kall[:, csl], ps[:, 0, :])
                    nc.scalar.copy(vall[:, csl], ps[:, 1, :])

            vT_tiles = []

            for jt in range(NJT):
                vt = pers.tile([JT, MEM + 1], BF16, tag=f"vt{jt}", name=f"vt{jt}")
                nc.gpsimd.memset(vt[:, MEM:MEM + 1], 1.0)
                vT_tiles.append(vt)

            def transpose_batch(j0, j1):
                for jt in range(j0, j1):
                    ps = mm.tile([JT, MEM], BF16, tag="mm", name=f"tp{jt}")
                    nc.tensor.transpose(
                        ps[:], vmem_bf[:, jt * JT:(jt + 1) * JT], ident[:]
                    )
                    nc.vector.tensor_copy(vT_tiles[jt][:, 0:MEM], ps[:])

            prologue_kv()
            transpose_batch(0, NJT // 2)   # prev half: only needs vmem DMA

            pctx.close()  # free prologue SBUF
            xin = ctx.enter_context(tc.tile_pool(name="xin", bufs=4))
            vhat = ctx.enter_context(tc.tile_pool(name="vhat", bufs=3))
            sev = ctx.enter_context(tc.tile_pool(name="sev", bufs=2))
            eloc = ctx.enter_context(tc.tile_pool(name="eloc", bufs=2))
            emem = ctx.enter_context(tc.tile_pool(name="emem", bufs=3))
            tree = ctx.enter_context(tc.tile_pool(name="tree", bufs=1))
            tail_p = ctx.enter_context(tc.tile_pool(name="tail", bufs=2))
            poolt = ctx.enter_context(tc.tile_pool(name="poolt", bufs=1))
            fin = ctx.enter_context(tc.tile_pool(name="fin", bufs=2))

            # ---- fused main loop ----
            HN = N // 2

            def mem_jtile(jt, h):
                sc = mm.tile([JT, 2, NB], F32, tag="mm", name=f"sc{jt}_{h}")
                for c in range(2):
                    nc.tensor.matmul(
                        sc[:, c, :],
                        kmem_bf[:, jt * JT:(jt + 1) * JT],
                        qp_bf[:, h * HN + c * NB:h * HN + (c + 1) * NB],
                        start=True, stop=True,
                    )
                e_m = emem.tile([JT, 2, NB], BF16, tag="em", name=f"em{jt}_{h}")
                nc.scalar.activation(e_m[:], sc[:], AF.Exp)
                for c in range(2):
                    nc.tensor.matmul(
                        num_ps_h[h][:, c * NB:(c + 1) * NB],
                        vT_tiles[jt][:],
                        e_m[:, c, :],
                        start=(jt == 0), stop=(jt == NJT - 1),
                    )

            out_view = out_d.rearrange("(m two) n -> m two n", two=2)

            def epilogue_chunk(c):
                csl = slice(c * NB, (c + 1) * NB)
                nc.vector.reciprocal_approx_fast(rden_m[:, csl], den_m[:, csl])
                ps = mm.tile([COUT, 2, NB], F32, tag="mm", name=f"ck{c}")
                nc.tensor.matmul(
                    ps[:, 0, :], wck_sb[:], num_bf[:, csl], start=True, stop=True
                )
                rb = fin.tile([COUT, NB], F32, tag="rb", name=f"rb{c}")
                nc.gpsimd.partition_broadcast(rb[:], rden_m[:, csl])
                t1 = fin.tile([COUT, NB], F32, tag="t1", name=f"t1{c}")
                nc.vector.tensor_tensor(t1[:], ps[:, 0, :], rb[:], op=ALU.mult)
                nc.vector.scalar_tensor_tensor(
                    t1[:], t1[:], gamma_sb[:], out_loc[:, csl],
                    op0=ALU.mult, op1=ALU.add,
                )
                nc.vector.tensor_scalar(
                    t1[:], t1[:], beta_sb[:], None, op0=ALU.add
                )
                nc.sync.dma_start(out_view[:, 0, csl], t1[0:MEM, :])
                nc.sync.dma_start(out_view[:, 1, csl], t1[MEM:COUT, :])

            fill_iter = iter([(jt, h) for h in range(2) for jt in range(NJT)])
            fill_state = {"done": 0}

            def evac_half(h):
                for c in range(2):
                    cg = h * 2 + c
                    nc.scalar.copy(
                        num_bf[:, cg * NB:(cg + 1) * NB],
                        num_psA[0:MEM, c * NB:(c + 1) * NB],
                    )
                    nc.scalar.copy(den_m[:, cg * NB:(cg + 1) * NB],
                                   num_psA[MEM:MEM + 1, c * NB:(c + 1) * NB])


            def fill(n):
                if FILL_DISABLE and n < 32:
                    return
                for _ in range(n):
                    nxt = next(fill_iter, None)
                    if nxt is not None:
                        mem_jtile(*nxt)
                        fill_state["done"] += 1
                        if fill_state["done"] in (NJT, 2 * NJT):
                            evac_half(fill_state["done"] // NJT - 1)

            for nb in range(NNB):
                nsl = slice(nb * NB, (nb + 1) * NB)
                x_g = [None] * KG
                s_sb = sev.tile([COUT, K, NB], BF16, tag="sev")
                e_sb = eloc.tile([COUT, K, NB], BF16, tag="e")
                t8 = tree.tile([COUT, 8, NB], BF16, tag="t8")
                d8 = tree.tile([COUT, 8, NB], BF16, tag="d8")

                def kproj(g):
                    x_g[g % KG] = xin.tile([CIN, KG, NB], BF16, tag="x",
                                           name=f"xg{nb}_{g}")
                    nc.gpsimd.dma_start(
                        x_g[g % KG][:], x_v[:, g * KG:(g + 1) * KG, nsl]
                    )
                    for h in range(2):
                        kk0 = g * KG + h * 2
                        p_i = kk0 // 2
                        ps = mm.tile([COUT, 2, NB], F32, tag="mm",
                                     name=f"kps{nb}{g}{h}")
                        for j2 in range(2):
                            nc.tensor.matmul(
                                ps[:, j2, :], wk_sb[:],
                                x_g[g % KG][:, h * 2 + j2, :],
                                start=True, stop=True,
                            )
                        for j2 in range(2):
                            kk = kk0 + j2
                            nc.tensor.matmul(
                                ps[:, j2, :],
                                relp_sb[:, kk * COUT:(kk + 1) * COUT],
                                ohpair[:],
                                start=False, stop=True, skip_group_check=True,
                            )
                        nc.vector.tensor_tensor(
                            s_sb[:, kk0:kk0 + 2, :], ps[:],
                            q_sb[:, nsl].rearrange('p (u n) -> p u n', u=1).broadcast_to([COUT, 2, NB]),
                            op=ALU.mult,
                        )

                def vproj(g):
                    ev = s_sb  # reuse: s is dead after exp
                    for h in range(2):
                        gi = g * 2 + h
                        kk0 = g * KG + h * 2
                        ps = mm.tile([COUT, 2, NB], F32, tag="mm",
                                     name=f"vps{nb}{g}{h}")
                        for j2 in range(2):
                            nc.tensor.matmul(
                                ps[:, j2, :], wv_sb[:],
                                x_g[g % KG][:, h * 2 + j2, :],
                                start=True, stop=True,
                            )
                        if gi % 4 == 0:
                            vh = vhat.tile([COUT, 2, NB], BF16, tag="vh",
                                           name=f"vh{nb}{g}{h}")
                            nc.scalar.copy(vh[:], ps[:])
                            nc.vector.tensor_tensor(
                                ev[:, kk0:kk0 + 2, :], e_sb[:, kk0:kk0 + 2, :],
                                vh[:], op=ALU.mult
                            )
                        else:
                            nc.vector.tensor_tensor(
                                ev[:, kk0:kk0 + 2, :], e_sb[:, kk0:kk0 + 2, :],
                                ps[:], op=ALU.mult
                            )

                ev = s_sb
                kproj(0)
                fill(1)
                kproj(1)
                nc.scalar.activation(e_sb[:, 0:8, :], s_sb[:, 0:8, :], AF.Exp)
                nc.vector.tensor_tensor(
                    d8[:, 0:4, :], e_sb[:, 0:4, :], e_sb[:, 4:8, :], op=ALU.add
                )
                fill(1)
                kproj(2)
                fill(1)
                kproj(3)
                nc.scalar.activation(e_sb[:, 8:16, :], s_sb[:, 8:16, :], AF.Exp)
                fill(1)
                vproj(0)
                fill(2)
                vproj(1)
                nc.vector.tensor_tensor(
                    t8[:, 0:4, :], ev[:, 0:4, :], ev[:, 4:8, :], op=ALU.add
                )
                fill(2)
                vproj(2)
                fill(2)
                vproj(3)
                fill(2)

                ev = s_sb
                nc.vector.tensor_tensor(
                    d8[:, 4:8, :], e_sb[:, 8:12, :], e_sb[:, 12:16, :], op=ALU.add
                )
                nc.vector.tensor_tensor(
                    t8[:, 4:8, :], ev[:, 8:12, :], ev[:, 12:16, :], op=ALU.add
                )
                nc.vector.tensor_tensor(
                    t8[:, 0:4, :], t8[:, 0:4, :], t8[:, 4:8, :], op=ALU.add
                )
                nc.vector.tensor_tensor(
                    d8[:, 0:4, :], d8[:, 0:4, :], d8[:, 4:8, :], op=ALU.add
                )
                nc.vector.tensor_tensor(
                    t8[:, 0:2, :], t8[:, 0:2, :], t8[:, 2:4, :], op=ALU.add
                )
                nc.vector.tensor_tensor(
                    d8[:, 0:2, :], d8[:, 0:2, :], d8[:, 2:4, :], op=ALU.add
                )
                numl = tail_p.tile([COUT, NB], F32, tag="numl")
                nc.vector.tensor_tensor(
                    numl[:], t8[:, 0, :], t8[:, 1, :], op=ALU.add
                )
                denl = tail_p.tile([COUT, NB], F32, tag="denl")
                nc.vector.tensor_tensor(
                    denl[:], d8[:, 0, :], d8[:, 1, :], op=ALU.add
                )
                rden = tail_p.tile([COUT, NB], F32, tag="rden")
                nc.vector.reciprocal_approx_fast(rden[:], denl[:])
                nc.vector.tensor_tensor(
                    out_loc[:, nsl], numl[:], rden[:], op=ALU.mult
                )
                if nb == 0:
                    khi = poolt.tile([MEM, N], BF16, tag="khi")
                    nc.sync.dma_start(khi[:], kall[MEM:COUT, :])
                    kpc = poolt.tile([MEM, N], BF16, tag="kpc")
                    nc.vector.tensor_tensor(kpc[:], kall[0:MEM, :], khi[:], op=ALU.max)
                    nc.vector.tensor_copy(kmem_bf[:, N:2 * N], kpc[:])
                    nc.gpsimd.dma_start(kp_d[:], kpc[:])  # cast bf16->f32
                    vhi = poolt.tile([MEM, N], BF16, tag="vhi")
                    nc.sync.dma_start(vhi[:], vall[MEM:COUT, :])
                    vpc = poolt.tile([MEM, N], BF16, tag="vpc")
                    nc.vector.tensor_tensor(vpc[:], vall[0:MEM, :], vhi[:], op=ALU.max)
                    nc.vector.tensor_copy(vmem_bf[:, N:2 * N], vpc[:])
                    nc.gpsimd.dma_start(vp_d[:], vpc[:])  # cast bf16->f32
                    transpose_batch(NJT // 2, NJT)

            fill(32)  # finish h=0 sweep (incl. evacuation)
            fill(64)  # h=1 sweep (incl. evacuation)
            for c in range(4):
                epilogue_chunk(c)



    nc.finalize()
    return nc


def _get_nc():
    if "nc" not in _CACHED:
        _CACHED["nc"] = build_nc()
    return _CACHED["nc"]


def kernel(x, abs_x, deg, idx, k, v, Wq, Wk, Wv, rel_k, Wck, bn_gamma, bn_beta,
           trace=False, tmpdir=None, trace_cores=None):
    x = np.asarray(x, dtype=np.float32)
    abs_x = np.asarray(abs_x, dtype=np.float32)
    k = np.asarray(k, dtype=np.float32)
    v = np.asarray(v, dtype=np.float32)
    Wq = np.asarray(Wq, dtype=np.float32)
    Wk = np.asarray(Wk, dtype=np.float32)
    Wv = np.asarray(Wv, dtype=np.float32)
    rel_k = np.asarray(rel_k, dtype=np.float32)
    Wck = np.asarray(Wck, dtype=np.float32)
    bn_gamma = np.asarray(bn_gamma, dtype=np.float32)
    bn_beta = np.asarray(bn_beta, dtype=np.float32)

    perm = np.concatenate([np.arange(0, COUT, 2), np.arange(1, COUT, 2)])
    bf = ml_dtypes.bfloat16

    wqT = np.ascontiguousarray(Wq[perm].T.astype(bf))
    wkT = np.ascontiguousarray(Wk[perm].T.astype(bf))
    wvT = np.ascontiguousarray(Wv[perm].T.astype(bf))
    rel = np.ascontiguousarray(rel_k[perm, 0, :].astype(np.float32))
    ohp = np.ones((1, 512), dtype=bf)
    relp = np.zeros((1, K * COUT), dtype=bf)
    for kk in range(K):
        relp[0, kk * COUT:(kk + 1) * COUT] = rel_k[perm, 0, kk].astype(bf)
    wckT = np.ascontiguousarray(Wck[perm].T.astype(bf))
    gamma = (bn_gamma * (1.0 / np.sqrt(1.0 + EPS)))[perm].reshape(COUT, 1)
    gamma = np.ascontiguousarray(gamma.astype(np.float32))
    beta = np.ascontiguousarray(bn_beta[perm].reshape(COUT, 1).astype(np.float32))

    x_t = np.ascontiguousarray(x.transpose(0, 1, 3, 2).reshape(B, CIN, K * N))
    absx_t = np.ascontiguousarray(abs_x[:, :, :, 0])
    k_t = np.ascontiguousarray(k[:, :, :, 0])
    v_t = np.ascontiguousarray(v[:, :, :, 0])

    nc = _get_nc()
    in_maps = []
    for b in range(B):
        in_maps.append({
            "x": x_t[b], "abs_x": absx_t[b], "kprev": k_t[b], "vprev": v_t[b],
            "wqT": wqT, "wkT": wkT, "wvT": wvT, "rel": rel, "relp": relp,
            "ohp": ohp, "wckT": wckT,
            "gamma": gamma, "beta": beta,
        })
    res = run_bass_kernel_spmd(
        nc, in_maps, core_ids=list(range(B)),
        trace=trace, tmpdir=tmpdir, trace_cores=trace_cores,
    )
    out = np.stack([res.results[b]["out"] for b in range(B)])[..., None]
    kp = np.stack([res.results[b]["kp"] for b in range(B)])[..., None]
    vp = np.stack([res.results[b]["vp"] for b in range(B)])[..., None]
    if trace:
        kernel.last_results = res
    return out, kp, vp

